# revision 30
# baseline (speedup 1.0000x reference)
"""GQA attention (B=2, S=2048, H=16, HKV=8, D=128) + RoPE + QKV/O proj
on 8 TRN2 NeuronCores.

Sharding: tensor-parallel by head. Core c computes QKV projection for its
2 q-heads / 1 kv-head over all tokens, RoPE, and full (non-causal)
attention for those heads.  An on-chip AllToAll then redistributes the
per-head attention outputs so core c holds *all* heads for its 512-token
block, and each core computes the output projection for its token block.
Host-side work is only shard/concat/dtype-cast (no arithmetic).

Matmul compute dtype: bfloat16 (fp32 PSUM accumulation).  End-to-end
rel-err vs the fp64 reference is ~2e-3, well inside the 2e-2 gate, and
bf16 halves DMA traffic, halves LDWEIGHTS bytes (enabling fast weight
load), and doubles DVE throughput for the elementwise work.

Schedule: each core owns 256 output tokens of each batch, so every
(local head, batch) quartet of attention groups feeds its own half-size
AllToAll that overlaps the next quartet's compute.  The output
projection runs as three waves that successively hide the collectives:
the odd-head pass (released by head 1's early A2As), then the even-head
batch-0 half (released by A2A(0,0)), then only the even-head batch-1
half trails the final 0.5 MB collective; odd-head partials spill to
SBUF and merge with the even-head waves on the vector engine.  w_o is
loaded once and stays resident across all waves.  Local head 1 runs
before head 0 so head 1's collectives overlap head 0's compute.
Q stays resident in SBUF (no DRAM bounce).  The exp activations on the
scalar engine (~1.25 us per [128,1024] tile, one engine) pace phase 2;
the softmax-denominator partial sums run as a bf16 running sum on the
vector engine, each add hiding behind its exp, so a single add remains
on each group's tail chain ahead of its AllToAll.

NOTE: a NEFF containing collectives runs the PE at ~2.0 GHz base clock
instead of the 2.4 GHz boost (measured: the same matmul stream paces
216 ns/MM without a collective in the program, 263 ns/MM with one), so
every matmul here pays ~22% over the boost-clock roofline; the AllToAll
is algorithmically required, making ~263 ns/MM the effective floor.
"""
import sys
import types

import numpy as np


def _install_ntff_hook():
    """The container's antenv stub lacks axon_hooks; shim it so
    run_bass_kernel_spmd(trace=True) can capture NTFF profiles."""
    try:
        import antenv.axon_hooks  # noqa: F401
        return
    except ImportError:
        pass
    try:
        import trn_agent_boot.trn_boot as tb
        hook = tb._ntff_profile_via_ctypes("/opt/axon/libaxon_pjrt.so")
        mod = types.ModuleType("antenv.axon_hooks")
        mod.get_axon_ntff_profile_hook = lambda: hook
        sys.modules["antenv.axon_hooks"] = mod
    except Exception:
        pass


_install_ntff_hook()

import concourse.mybir as mybir  # noqa: E402
import concourse.tile as tile  # noqa: E402
from concourse import bacc  # noqa: E402
from concourse.bass_utils import run_bass_kernel_spmd  # noqa: E402

F32 = mybir.dt.float32
F32R = mybir.dt.float32r
BF16 = mybir.dt.bfloat16
FP8 = mybir.dt.float8e4
DR = mybir.MatmulPerfMode.DoubleRow
AF = mybir.ActivationFunctionType
NP_BF16 = mybir.dt.np(mybir.dt.bfloat16)

B, S, HID = 2, 2048, 2048
H, HKV, D = 16, 8, 128
NCORES = 8
TOK = B * S              # 4096 stacked tokens (batch-major)
TPC = TOK // NCORES      # 512 tokens owned per core
KCH = HID // 128         # 16 contraction chunks
SCALE = 1.0 / float(np.sqrt(D))


def build():
    nc = bacc.Bacc("TRN2", target_bir_lowering=False, debug=False,
                   num_devices=NCORES)

    hT = nc.dram_tensor("hT", [HID, TOK], BF16, kind="ExternalInput")
    wqT = nc.dram_tensor("wqT", [HID, 4 * D], BF16, kind="ExternalInput")
    woT = nc.dram_tensor("woT", [H * D, HID], BF16, kind="ExternalInput")
    cosT = nc.dram_tensor("cosT", [D, TOK], BF16, kind="ExternalInput")
    sinST = nc.dram_tensor("sinST", [D, TOK], BF16, kind="ExternalInput")
    ones_col = nc.dram_tensor("ones_col", [128, 1], BF16, kind="ExternalInput")
    ones_row = nc.dram_tensor("ones_row", [1, 128], BF16, kind="ExternalInput")
    ident = nc.dram_tensor("ident", [128, 128], BF16, kind="ExternalInput")
    out = nc.dram_tensor("out", [TPC, HID], F32, kind="ExternalOutput")

    hT_v = hT[:].rearrange("(c p) t -> p c t", p=128)      # [128, 16, 4096]
    wqT_v = wqT[:].rearrange("(c p) m -> p c m", p=128)    # [128, 16, 512]
    woT_v = woT[:].rearrange("(c p) o -> p c o", p=128)    # [128, 16, 2048]

    with tile.TileContext(nc) as tc:
        with (
            tc.tile_pool(name="cst", bufs=1) as cst,
            tc.tile_pool(name="res", bufs=1) as res,
            tc.tile_pool(name="med", bufs=32) as med,
            tc.tile_pool(name="tbl", bufs=3) as tblp,
            tc.tile_pool(name="wk", bufs=10) as wkp,
            tc.tile_pool(name="pp", bufs=8) as ppp,
            tc.tile_pool(name="rr", bufs=2) as rrp,
            tc.tile_pool(name="aa", bufs=1) as aap,
            tc.tile_pool(name="psS", bufs=2, space="PSUM") as psS,
            tc.tile_pool(name="psacc", bufs=3, space="PSUM") as psacc,
            tc.tile_pool(name="psden", bufs=1, space="PSUM") as psden,
            tc.tile_pool(name="dram", bufs=1, space="DRAM") as dram,
        ):
            # ---- constants ----
            ident_s = cst.tile([128, 128], BF16, name="ident_s")
            ones_c = cst.tile([128, 1], BF16, name="ones_c")
            ones_r = cst.tile([1, 128], BF16, name="ones_r")
            nc.sync.dma_start(ident_s[:], ident[:])
            nc.sync.dma_start(ones_c[:], ones_col[:])
            nc.sync.dma_start(ones_r[:], ones_row[:])

            # ---- resident tensors (per-partition: 16+8+8+16 = 48 KB) ----
            wq_s = res.tile([128, KCH, 4 * D], BF16, name="wq_s")
            kT = res.tile([128, TOK], BF16, name="kT")
            V = res.tile([128, 32, 128], BF16, name="V")
            qsb = res.tile([128, 2, TOK], BF16, name="qsb")
            # odd-head output-projection partials (16 KB/partition)
            oev = res.tile([128, 16, 512], BF16, name="oev")

            # A2A DRAM bounces: one collective per (local head, batch).
            # Each core owns 256 tokens of each batch, so every 4-group
            # (head, batch) quartet feeds a half-size AllToAll that
            # overlaps the next quartet's compute; only the last 0.5 MB
            # collective is exposed.
            a2a_in = [
                [dram.tile([NCORES, 128, 256], BF16, name=f"a2a_in{h}_{b}")
                 for b in range(2)]
                for h in range(2)
            ]
            a2a_out = [
                [dram.tile([NCORES, 128, 256], BF16, name=f"a2a_out{h}_{b}")
                 for b in range(2)]
                for h in range(2)
            ]

            # ================= Phase 1: QKV + RoPE =================
            def ph1_block(tb):
                t0 = tb * 512
                hid_t = []
                for hq in range(8):
                    if tb == 0:
                        nc.sync.dma_start(wq_s[:, 2 * hq, :],
                                          wqT_v[:, 2 * hq, :])
                        nc.sync.dma_start(wq_s[:, 2 * hq + 1, :],
                                          wqT_v[:, 2 * hq + 1, :])
                    ht_ = med.tile([128, 2, 512], BF16, tag="med",
                                   name=f"hid{hq}")
                    if tb <= 1:
                        # startup: two half-size DMAs so the first chunk
                        # clears the shared-bandwidth burst sooner
                        nc.sync.dma_start(
                            ht_[:, 0, :], hT_v[:, 2 * hq, t0:t0 + 512])
                        nc.sync.dma_start(
                            ht_[:, 1, :], hT_v[:, 2 * hq + 1, t0:t0 + 512])
                    else:
                        nc.sync.dma_start(
                            ht_[:], hT_v[:, 2 * hq:2 * hq + 2, t0:t0 + 512])
                    hid_t.append(ht_)
                cosc = tblp.tile([128, 512], BF16, tag="cosc", name="cosc")
                sinc = tblp.tile([128, 512], BF16, tag="sinc", name="sinc")
                nc.sync.dma_start(cosc[:], cosT[:, t0:t0 + 512])
                nc.sync.dma_start(sinc[:], sinST[:, t0:t0 + 512])

                def qkv_epilogue(m, ps):
                    if m == 3:
                        # V: evict (bf16) then PE-transpose to [t, d]
                        vt = wkp.tile([128, 512], BF16, tag="wk", name="vt")
                        with nc.allow_low_precision(reason="bf16 compute"):
                            nc.vector.tensor_copy(vt[:], ps[:])
                        for i in range(4):
                            trp = psden.tile([128, 128], BF16, tag="ps_d",
                                             name="ps_tr")
                            nc.tensor.matmul(
                                trp[:], vt[:, i * 128:(i + 1) * 128],
                                ident_s[:], is_transpose=True,
                            )
                            nc.vector.tensor_copy(V[:, tb * 4 + i, :],
                                                  trp[:])
                    else:
                        # q/k RoPE:  dest = ps*cos + swap(ps)*sinS
                        if m == 2:
                            dest = kT[:, t0:t0 + 512]
                        else:
                            dest = qsb[:, m, t0:t0 + 512]
                        qraw = wkp.tile([128, 512], BF16, tag="wk",
                                        name="qraw")
                        rot = wkp.tile([128, 512], BF16, tag="wk",
                                       name="rot")
                        tmp = wkp.tile([128, 512], BF16, tag="wk",
                                       name="tmp")
                        with nc.allow_low_precision(reason="bf16 compute"):
                            nc.scalar.copy(qraw[:], ps[:])
                        nc.vector.tensor_mul(tmp[:], qraw[:], cosc[:])
                        nc.sync.dma_start(rot[0:64, :], qraw[64:128, :])
                        nc.sync.dma_start(rot[64:128, :], qraw[0:64, :])
                        nc.vector.tensor_mul(rot[:], rot[:], sinc[:])
                        nc.vector.tensor_add(dest, tmp[:], rot[:])

                if tb == 0:
                    # Startup block: kk-outer so the first matmuls depend
                    # only on the first wq/hid DMA chunks rather than the
                    # whole ~4.7 MB startup burst (all four m-accumulators
                    # open at once: three psacc banks + the psden bank).
                    pss = [psacc.tile([128, 512], F32, tag="ps_a",
                                      name=f"ps_qkv{m}") for m in range(3)]
                    pss.append(psden.tile([128, 512], F32, tag="ps_d",
                                          name="ps_qkv3"))
                    for kk in range(KCH):
                        for m in range(4):
                            nc.tensor.matmul(
                                pss[m][:],
                                wq_s[:, kk, m * 128:(m + 1) * 128],
                                hid_t[kk // 2][:, kk % 2, :],
                                start=(kk == 0), stop=(kk == KCH - 1),
                            )
                    for m in range(4):
                        qkv_epilogue(m, pss[m])
                else:
                    for m in range(4):  # q0, q1, k, v
                        ps = psacc.tile([128, 512], F32, tag="ps_a",
                                        name="ps_qkv")
                        for kk in range(KCH):
                            nc.tensor.matmul(
                                ps[:],
                                wq_s[:, kk, m * 128:(m + 1) * 128],
                                hid_t[kk // 2][:, kk % 2, :],
                                start=(kk == 0), stop=(kk == KCH - 1),
                            )
                        qkv_epilogue(m, ps)

            # ================= Phase 2: attention =================
            def ph2_group(h, b, qb):
                q0 = b * S + qb * 512
                qsl = qsb[:, h, q0:q0 + 512]
                psA = psacc.tile([128, 512], F32, tag="ps_a", name="ps_av")
                tA = rrp.tile([128, 1024], BF16, tag="rrA", name="tA")
                Ps = []
                for i in range(8):  # pairs of 128-token k/v blocks
                    ta = b * S + (2 * i) * 128
                    tbk = b * S + (2 * i + 1) * 128
                    psSt = psS.tile([128, 1024], F32, tag="ps_s",
                                    name="ps_sc")
                    nc.tensor.matmul(psSt[:, 0:512],
                                     kT[:, ta:ta + 128], qsl,
                                     start=True, stop=True)
                    nc.tensor.matmul(psSt[:, 512:1024],
                                     kT[:, tbk:tbk + 128], qsl,
                                     start=True, stop=True)
                    P = ppp.tile([128, 1024], BF16, tag="pp", name="P")
                    nc.scalar.activation(P[:], psSt[:], AF.Exp, scale=SCALE)
                    nc.tensor.matmul(psA[:], V[:, b * 16 + 2 * i, :],
                                     P[:, 0:512],
                                     start=(i == 0), stop=False)
                    nc.tensor.matmul(psA[:], V[:, b * 16 + 2 * i + 1, :],
                                     P[:, 512:1024],
                                     start=False, stop=(i == 7))
                    Ps.append(P)
                    # bf16 running sum of exp tiles on DVE: each add
                    # lands right after its exp, so only one add remains
                    # on the group's tail chain
                    if i == 1:
                        nc.vector.tensor_add(tA[:], Ps[0][:], Ps[1][:])
                    elif i >= 2:
                        nc.vector.tensor_add(tA[:], tA[:], P[:])
                # denominator: fold 1024->512, cross-partition sum via a
                # ones-vector matmul, reciprocal, rank-1 matmul broadcast
                R2 = wkp.tile([128, 512], BF16, tag="wk", name="R2")
                nc.vector.tensor_add(R2[:], tA[:, 0:512], tA[:, 512:1024])
                psD = psden.tile([1, 512], F32, tag="ps_d", name="ps_den")
                nc.tensor.matmul(psD[:], ones_c[:], R2[:],
                                 start=True, stop=True)
                rec32 = wkp.tile([1, 512], F32, tag="wk", name="rec32")
                nc.vector.reciprocal_approx_fast(rec32[:], psD[:])
                rec = wkp.tile([1, 512], BF16, tag="wk", name="rec")
                with nc.allow_low_precision(
                        reason="bf16 rounding of softmax denom"):
                    nc.vector.tensor_copy(rec[:], rec32[:])
                psB = psden.tile([128, 512], F32, tag="ps_d", name="ps_bc")
                nc.tensor.matmul(psB[:], ones_r[:], rec[:],
                                 start=True, stop=True)
                bcB = wkp.tile([128, 512], F32, tag="wk", name="bcB")
                nc.vector.tensor_copy(bcB[:], psB[:])
                attn = wkp.tile([128, 512], BF16, tag="wk", name="attn")
                with nc.allow_low_precision(reason="bf16 attn out"):
                    nc.vector.tensor_mul(attn[:], psA[:], bcB[:])
                nc.sync.dma_start(a2a_in[h][b][2 * qb], attn[:, 0:256])
                nc.sync.dma_start(a2a_in[h][b][2 * qb + 1], attn[:, 256:512])

            # ===== schedule: ph1 b0, then interleave ph1 b1 with head-1
            # attention on b0; head-1 finishes first so its A2A overlaps
            # head-0 attention; the final A2A overlaps ph3's odd pass.
            for tb in range(4):
                ph1_block(tb)
            def a2a(h, b):
                nc.gpsimd.collective_compute(
                    "AllToAll", mybir.AluOpType.bypass,
                    replica_groups=[list(range(NCORES))],
                    ins=[a2a_in[h][b].opt()], outs=[a2a_out[h][b].opt()],
                )

            for qb in range(4):
                ph1_block(4 + qb)
                ph2_group(1, 0, qb)
            a2a(1, 0)
            for qb in range(4):
                ph2_group(1, 1, qb)
            a2a(1, 1)
            att_od = aap.tile([128, 8, 512], BF16, name="att_od")
            for j in range(NCORES):
                nc.gpsimd.dma_start(att_od[:, j, 0:256], a2a_out[1][0][j])
                nc.gpsimd.dma_start(att_od[:, j, 256:512], a2a_out[1][1][j])
            for qb in range(4):
                ph2_group(0, 0, qb)
            a2a(0, 0)
            for qb in range(4):
                ph2_group(0, 1, qb)
            a2a(0, 1)
            att_ev = aap.tile([128, 8, 512], BF16, name="att_ev")
            for j in range(NCORES):
                nc.sync.dma_start(att_ev[:, j, 0:256], a2a_out[0][0][j])
                nc.sync.dma_start(att_ev[:, j, 256:512], a2a_out[0][1][j])

            # ================= Phase 3: output projection =================
            # Pass 1 (odd heads, from the early A2A) runs while the final
            # A2A is in flight; partials spill to SBUF.  Pass 2 adds the
            # even heads and stores.
            def wo_load(n):
                n0 = n * 512
                tiles = []
                for wq4 in range(8):
                    wt_ = med.tile([128, 2, 512], BF16, tag="med",
                                   name=f"wo{wq4}")
                    nc.sync.dma_start(
                        wt_[:], woT_v[:, 2 * wq4:2 * wq4 + 2, n0:n0 + 512])
                    tiles.append(wt_)
                return tiles

            def half_mms(psO, att, kh0, wo_t, m):
                for j in range(8):
                    kh = 2 * j + kh0
                    nc.tensor.matmul(
                        psO[:],
                        att[:, j, m * 128:(m + 1) * 128],
                        wo_t[kh // 2][:, kh % 2, :],
                        start=(j == 0), stop=(j == 7),
                    )

            wo_ts = []
            for n in range(4):
                wo_t = wo_load(n)
                wo_ts.append(wo_t)
                for m in range(4):
                    pool, tag = ((psacc, "ps_a") if m % 2 == 0 else
                                 (psS, "ps_s"))
                    psO = pool.tile([128, 512], F32, tag=tag, name="ps_o")
                    half_mms(psO, att_od, 1, wo_t, m)
                    with nc.allow_low_precision(reason="bf16 partials"):
                        nc.scalar.copy(oev[:, 4 * n + m, :], psO[:])
            # Even-head pass split by batch-half: m=0,1 token columns
            # arrive with A2A(0,0) one collective earlier, so that half
            # releases mid-attention and only m=2,3 trails the final A2A.
            # (Viable now that w_o stays resident — the v11 attempt died
            # on reload-DMA pacing.)
            for half in range(2):
                for n in range(4):
                    wo_t = wo_ts[n]
                    oevfs = {}
                    for m in (2 * half, 2 * half + 1):
                        oevf = wkp.tile([128, 512], F32, tag="wk",
                                        name="oevf")
                        nc.scalar.copy(oevf[:], oev[:, 4 * n + m, :])
                        oevfs[m] = oevf
                    for m in (2 * half, 2 * half + 1):
                        pool, tag = ((psacc, "ps_a") if m % 2 == 0 else
                                     (psS, "ps_s"))
                        psO = pool.tile([128, 512], F32, tag=tag,
                                        name="ps_o")
                        half_mms(psO, att_ev, 0, wo_t, m)
                        oout = wkp.tile([128, 512], F32, tag="wk",
                                        name="oout")
                        nc.vector.tensor_add(oout[:], psO[:],
                                             oevfs[m][:])
                        nc.sync.dma_start(out[m * 128:(m + 1) * 128,
                                              n * 512:(n + 1) * 512],
                                          oout[:])

    nc.compile()
    return nc


def shard_inputs(cos, sin, hidden_states, w_qkv, w_o):
    """Host-side resharding into per-core input maps (pure data movement +
    layout transposes + dtype casts; no arithmetic on the model data)."""
    hs = np.ascontiguousarray(hidden_states.astype(np.float32))
    hT = np.ascontiguousarray(hs.reshape(TOK, HID).T).astype(NP_BF16)
    cosTt = np.ascontiguousarray(
        np.tile(cos.astype(np.float32).T, (1, B))).astype(NP_BF16)
    sT = sin.astype(np.float32).T                               # [128, 2048]
    sinST = np.concatenate([-sT[:64], sT[64:]], axis=0)
    sinST = np.ascontiguousarray(
        np.tile(sinST, (1, B))).astype(NP_BF16)                 # [128, 4096]
    woT = np.ascontiguousarray(w_o.astype(np.float32).T).astype(NP_BF16)
    ident = np.eye(128, dtype=np.float32).astype(NP_BF16)
    ones_col = np.ones((128, 1), np.float32).astype(NP_BF16)
    ones_row = np.ones((1, 128), np.float32).astype(NP_BF16)

    in_maps = []
    for c in range(NCORES):
        rows = [w_qkv[2 * c * D:(2 * c + 2) * D],
                w_qkv[(H + c) * D:(H + c + 1) * D],
                w_qkv[(H + HKV + c) * D:(H + HKV + c + 1) * D]]
        wq_c = np.concatenate(rows, axis=0).astype(np.float32)  # [512, 2048]
        wqT_c = np.ascontiguousarray(wq_c.T).astype(NP_BF16)    # [2048, 512]
        in_maps.append({
            "hT": hT, "wqT": wqT_c, "woT": woT,
            "cosT": cosTt, "sinST": sinST, "ident": ident,
            "ones_col": ones_col, "ones_row": ones_row,
        })
    return in_maps


_cached_nc = None


def kernel(cos, sin, hidden_states, w_qkv, w_o, _trace=False):
    global _cached_nc
    if _cached_nc is None:
        _cached_nc = build()
    nc = _cached_nc
    in_maps = shard_inputs(cos, sin, hidden_states, w_qkv, w_o)
    res = run_bass_kernel_spmd(nc, in_maps, core_ids=list(range(NCORES)),
                               trace=_trace)
    full = np.empty((B, S, HID), np.float32)
    for c in range(NCORES):
        p = res.results[c]["out"]
        full[0, 256 * c:256 * c + 256] = p[0:256]
        full[1, 256 * c:256 * c + 256] = p[256:512]
    out = np.ascontiguousarray(full)
    if _trace:
        return out, res
    return out


# revision 32
# speedup vs baseline: 1.2395x; 1.2395x over previous
"""GQA attention (B=2, S=2048, H=16, HKV=8, D=128) + RoPE + QKV/O proj
on 8 TRN2 NeuronCores.

Sharding: tensor-parallel by head. Core c computes QKV projection for its
2 q-heads / 1 kv-head over all tokens, RoPE, and full (non-causal)
attention for those heads.  An on-chip AllToAll then redistributes the
per-head attention outputs so core c holds *all* heads for its 512-token
block, and each core computes the output projection for its token block.
Host-side work is only shard/concat/dtype-cast (no arithmetic).

Matmul compute dtype: bfloat16 (fp32 PSUM accumulation).  End-to-end
rel-err vs the fp64 reference is ~2e-3, well inside the 2e-2 gate, and
bf16 halves DMA traffic, halves LDWEIGHTS bytes (enabling fast weight
load), and doubles DVE throughput for the elementwise work.

Schedule: each core owns 256 output tokens of each batch, so every
(local head, batch) quartet of attention groups feeds its own half-size
AllToAll that overlaps the next quartet's compute.  The output
projection runs as three waves that successively hide the collectives:
the odd-head pass (released by head 1's early A2As), then the even-head
batch-0 half (released by A2A(0,0)), then only the even-head batch-1
half trails the final 0.5 MB collective; odd-head partials spill to
SBUF and merge with the even-head waves on the vector engine.  w_o is
loaded once and stays resident across all waves.  Local head 1 runs
before head 0 so head 1's collectives overlap head 0's compute.
Q stays resident in SBUF (no DRAM bounce).  The exp activations on the
scalar engine (~1.25 us per [128,1024] tile, one engine) pace phase 2;
the softmax-denominator partial sums run as a bf16 running sum on the
vector engine, each add hiding behind its exp, so a single add remains
on each group's tail chain ahead of its AllToAll.

NOTE: a NEFF containing collectives runs the PE at ~2.0 GHz base clock
instead of the 2.4 GHz boost (measured: the same matmul stream paces
216 ns/MM without a collective in the program, 263 ns/MM with one), so
every matmul here pays ~22% over the boost-clock roofline; the AllToAll
is algorithmically required, making ~263 ns/MM the effective floor.
"""
import sys
import types

import numpy as np


def _install_ntff_hook():
    """The container's antenv stub lacks axon_hooks; shim it so
    run_bass_kernel_spmd(trace=True) can capture NTFF profiles."""
    try:
        import antenv.axon_hooks  # noqa: F401
        return
    except ImportError:
        pass
    try:
        import trn_agent_boot.trn_boot as tb
        hook = tb._ntff_profile_via_ctypes("/opt/axon/libaxon_pjrt.so")
        mod = types.ModuleType("antenv.axon_hooks")
        mod.get_axon_ntff_profile_hook = lambda: hook
        sys.modules["antenv.axon_hooks"] = mod
    except Exception:
        pass


_install_ntff_hook()

import concourse.mybir as mybir  # noqa: E402
import concourse.tile as tile  # noqa: E402
from concourse import bacc  # noqa: E402
from concourse.bass_utils import run_bass_kernel_spmd  # noqa: E402

F32 = mybir.dt.float32
F32R = mybir.dt.float32r
BF16 = mybir.dt.bfloat16
FP8 = mybir.dt.float8e4
DR = mybir.MatmulPerfMode.DoubleRow
AF = mybir.ActivationFunctionType
NP_BF16 = mybir.dt.np(mybir.dt.bfloat16)

B, S, HID = 2, 2048, 2048
H, HKV, D = 16, 8, 128
NCORES = 8
TOK = B * S              # 4096 stacked tokens (batch-major)
TPC = TOK // NCORES      # 512 tokens owned per core
KCH = HID // 128         # 16 contraction chunks
SCALE = 1.0 / float(np.sqrt(D))


def build():
    nc = bacc.Bacc("TRN2", target_bir_lowering=False, debug=False,
                   num_devices=NCORES)

    hT = nc.dram_tensor("hT", [HID, TOK], BF16, kind="ExternalInput")
    wqT = nc.dram_tensor("wqT", [HID, 4 * D], BF16, kind="ExternalInput")
    woT = nc.dram_tensor("woT", [H * D, HID], BF16, kind="ExternalInput")
    cosT = nc.dram_tensor("cosT", [D, TOK], BF16, kind="ExternalInput")
    sinST = nc.dram_tensor("sinST", [D, TOK], BF16, kind="ExternalInput")
    ones_col = nc.dram_tensor("ones_col", [128, 1], BF16, kind="ExternalInput")
    ones_row = nc.dram_tensor("ones_row", [1, 128], BF16, kind="ExternalInput")
    ident = nc.dram_tensor("ident", [128, 128], BF16, kind="ExternalInput")
    out = nc.dram_tensor("out", [TPC, HID], F32, kind="ExternalOutput")

    hT_v = hT[:].rearrange("(c p) t -> p c t", p=128)      # [128, 16, 4096]
    wqT_v = wqT[:].rearrange("(c p) m -> p c m", p=128)    # [128, 16, 512]
    woT_v = woT[:].rearrange("(c p) o -> p c o", p=128)    # [128, 16, 2048]

    with tile.TileContext(nc) as tc:
        with (
            tc.tile_pool(name="cst", bufs=1) as cst,
            tc.tile_pool(name="res", bufs=1) as res,
            tc.tile_pool(name="med", bufs=32) as med,
            tc.tile_pool(name="tbl", bufs=3) as tblp,
            tc.tile_pool(name="wk", bufs=10) as wkp,
            tc.tile_pool(name="pp", bufs=8) as ppp,
            tc.tile_pool(name="rr", bufs=2) as rrp,
            tc.tile_pool(name="aa", bufs=1) as aap,
            tc.tile_pool(name="psS", bufs=2, space="PSUM") as psS,
            tc.tile_pool(name="psacc", bufs=3, space="PSUM") as psacc,
            tc.tile_pool(name="psden", bufs=1, space="PSUM") as psden,
            tc.tile_pool(name="dram", bufs=1, space="DRAM") as dram,
        ):
            # ---- constants ----
            ident_s = cst.tile([128, 128], BF16, name="ident_s")
            ones_c = cst.tile([128, 1], BF16, name="ones_c")
            ones_r = cst.tile([1, 128], BF16, name="ones_r")
            nc.sync.dma_start(ident_s[:], ident[:])
            nc.sync.dma_start(ones_c[:], ones_col[:])
            nc.sync.dma_start(ones_r[:], ones_row[:])

            # ---- resident tensors (per-partition: 16+8+8+16 = 48 KB) ----
            wq_s = res.tile([128, KCH, 4 * D], BF16, name="wq_s")
            kT = res.tile([128, TOK], BF16, name="kT")
            V = res.tile([128, 32, 128], BF16, name="V")
            qsb = res.tile([128, 2, TOK], BF16, name="qsb")
            # odd-head output-projection partials (16 KB/partition)
            oev = res.tile([128, 16, 512], BF16, name="oev")

            # A2A DRAM bounces: one collective per (local head, batch).
            # Each core owns 256 tokens of each batch, so every 4-group
            # (head, batch) quartet feeds a half-size AllToAll that
            # overlaps the next quartet's compute; only the last 0.5 MB
            # collective is exposed.
            a2a_in = [
                [dram.tile([NCORES, 128, 256], BF16, name=f"a2a_in{h}_{b}")
                 for b in range(2)]
                for h in range(2)
            ]
            a2a_out = [
                [dram.tile([NCORES, 128, 256], BF16, name=f"a2a_out{h}_{b}")
                 for b in range(2)]
                for h in range(2)
            ]

            # ================= Phase 1: QKV + RoPE =================
            def ph1_block(tb):
                t0 = tb * 512
                hid_t = []
                for hq in range(8):
                    if tb == 0:
                        nc.sync.dma_start(wq_s[:, 2 * hq, :],
                                          wqT_v[:, 2 * hq, :])
                        nc.sync.dma_start(wq_s[:, 2 * hq + 1, :],
                                          wqT_v[:, 2 * hq + 1, :])
                    ht_ = med.tile([128, 2, 512], BF16, tag="med",
                                   name=f"hid{hq}")
                    if tb <= 1:
                        # startup: two half-size DMAs so the first chunk
                        # clears the shared-bandwidth burst sooner
                        nc.sync.dma_start(
                            ht_[:, 0, :], hT_v[:, 2 * hq, t0:t0 + 512])
                        nc.sync.dma_start(
                            ht_[:, 1, :], hT_v[:, 2 * hq + 1, t0:t0 + 512])
                    else:
                        nc.sync.dma_start(
                            ht_[:], hT_v[:, 2 * hq:2 * hq + 2, t0:t0 + 512])
                    hid_t.append(ht_)
                cosc = tblp.tile([128, 512], BF16, tag="cosc", name="cosc")
                sinc = tblp.tile([128, 512], BF16, tag="sinc", name="sinc")
                nc.sync.dma_start(cosc[:], cosT[:, t0:t0 + 512])
                nc.sync.dma_start(sinc[:], sinST[:, t0:t0 + 512])

                def qkv_epilogue(m, ps):
                    if m == 3:
                        # V: evict (bf16) then PE-transpose to [t, d]
                        vt = wkp.tile([128, 512], BF16, tag="wk", name="vt")
                        with nc.allow_low_precision(reason="bf16 compute"):
                            nc.vector.tensor_copy(vt[:], ps[:])
                        for i in range(4):
                            trp = psden.tile([128, 128], BF16, tag="ps_d",
                                             name="ps_tr")
                            nc.tensor.matmul(
                                trp[:], vt[:, i * 128:(i + 1) * 128],
                                ident_s[:], is_transpose=True,
                            )
                            nc.vector.tensor_copy(V[:, tb * 4 + i, :],
                                                  trp[:])
                    else:
                        # q/k RoPE:  dest = ps*cos + swap(ps)*sinS
                        if m == 2:
                            dest = kT[:, t0:t0 + 512]
                        else:
                            dest = qsb[:, m, t0:t0 + 512]
                        qraw = wkp.tile([128, 512], BF16, tag="wk",
                                        name="qraw")
                        rot = wkp.tile([128, 512], BF16, tag="wk",
                                       name="rot")
                        tmp = wkp.tile([128, 512], BF16, tag="wk",
                                       name="tmp")
                        with nc.allow_low_precision(reason="bf16 compute"):
                            nc.scalar.copy(qraw[:], ps[:])
                        nc.vector.tensor_mul(tmp[:], qraw[:], cosc[:])
                        nc.sync.dma_start(rot[0:64, :], qraw[64:128, :])
                        nc.sync.dma_start(rot[64:128, :], qraw[0:64, :])
                        nc.vector.tensor_mul(rot[:], rot[:], sinc[:])
                        nc.vector.tensor_add(dest, tmp[:], rot[:])

                if tb == 0:
                    # Startup block: kk-outer so the first matmuls depend
                    # only on the first wq/hid DMA chunks rather than the
                    # whole ~4.7 MB startup burst (all four m-accumulators
                    # open at once: three psacc banks + the psden bank).
                    pss = [psacc.tile([128, 512], F32, tag="ps_a",
                                      name=f"ps_qkv{m}") for m in range(3)]
                    pss.append(psden.tile([128, 512], F32, tag="ps_d",
                                          name="ps_qkv3"))
                    for kk in range(KCH):
                        for m in range(4):
                            nc.tensor.matmul(
                                pss[m][:],
                                wq_s[:, kk, m * 128:(m + 1) * 128],
                                hid_t[kk // 2][:, kk % 2, :],
                                start=(kk == 0), stop=(kk == KCH - 1),
                            )
                    for m in range(4):
                        qkv_epilogue(m, pss[m])
                else:
                    for m in range(4):  # q0, q1, k, v
                        ps = psacc.tile([128, 512], F32, tag="ps_a",
                                        name="ps_qkv")
                        for kk in range(KCH):
                            nc.tensor.matmul(
                                ps[:],
                                wq_s[:, kk, m * 128:(m + 1) * 128],
                                hid_t[kk // 2][:, kk % 2, :],
                                start=(kk == 0), stop=(kk == KCH - 1),
                            )
                        qkv_epilogue(m, ps)

            # ================= Phase 2: attention =================
            def ph2_group(h, b, qb):
                q0 = b * S + qb * 512
                qsl = qsb[:, h, q0:q0 + 512]
                psA = psacc.tile([128, 512], F32, tag="ps_a", name="ps_av")
                tA = rrp.tile([128, 1024], BF16, tag="rrA", name="tA")
                Ps = []
                for i in range(8):  # pairs of 128-token k/v blocks
                    ta = b * S + (2 * i) * 128
                    tbk = b * S + (2 * i + 1) * 128
                    psSt = psS.tile([128, 1024], F32, tag="ps_s",
                                    name="ps_sc")
                    nc.tensor.matmul(psSt[:, 0:512],
                                     kT[:, ta:ta + 128], qsl,
                                     start=True, stop=True)
                    nc.tensor.matmul(psSt[:, 512:1024],
                                     kT[:, tbk:tbk + 128], qsl,
                                     start=True, stop=True)
                    P = ppp.tile([128, 1024], BF16, tag="pp", name="P")
                    nc.scalar.activation(P[:], psSt[:], AF.Exp, scale=SCALE)
                    nc.tensor.matmul(psA[:], V[:, b * 16 + 2 * i, :],
                                     P[:, 0:512],
                                     start=(i == 0), stop=False)
                    nc.tensor.matmul(psA[:], V[:, b * 16 + 2 * i + 1, :],
                                     P[:, 512:1024],
                                     start=False, stop=(i == 7))
                    Ps.append(P)
                    # bf16 running sum of exp tiles on DVE: each add
                    # lands right after its exp, so only one add remains
                    # on the group's tail chain
                    if i == 1:
                        nc.vector.tensor_add(tA[:], Ps[0][:], Ps[1][:])
                    elif i >= 2:
                        nc.vector.tensor_add(tA[:], tA[:], P[:])
                # denominator: fold 1024->512, cross-partition sum via a
                # ones-vector matmul, reciprocal, rank-1 matmul broadcast
                R2 = wkp.tile([128, 512], BF16, tag="wk", name="R2")
                nc.vector.tensor_add(R2[:], tA[:, 0:512], tA[:, 512:1024])
                psD = psden.tile([1, 512], F32, tag="ps_d", name="ps_den")
                nc.tensor.matmul(psD[:], ones_c[:], R2[:],
                                 start=True, stop=True)
                rec32 = wkp.tile([1, 512], F32, tag="wk", name="rec32")
                nc.vector.reciprocal_approx_fast(rec32[:], psD[:])
                rec = wkp.tile([1, 512], BF16, tag="wk", name="rec")
                with nc.allow_low_precision(
                        reason="bf16 rounding of softmax denom"):
                    nc.vector.tensor_copy(rec[:], rec32[:])
                psB = psden.tile([128, 512], F32, tag="ps_d", name="ps_bc")
                nc.tensor.matmul(psB[:], ones_r[:], rec[:],
                                 start=True, stop=True)
                bcB = wkp.tile([128, 512], F32, tag="wk", name="bcB")
                nc.vector.tensor_copy(bcB[:], psB[:])
                attn = wkp.tile([128, 512], BF16, tag="wk", name="attn")
                with nc.allow_low_precision(reason="bf16 attn out"):
                    nc.vector.tensor_mul(attn[:], psA[:], bcB[:])
                nc.sync.dma_start(a2a_in[h][b][2 * qb], attn[:, 0:256])
                nc.sync.dma_start(a2a_in[h][b][2 * qb + 1], attn[:, 256:512])

            # ===== schedule: ph1 b0, then interleave ph1 b1 with head-1
            # attention on b0; head-1 finishes first so its A2A overlaps
            # head-0 attention; the final A2A overlaps ph3's odd pass.
            for tb in range(4):
                ph1_block(tb)
            def a2a(h, b):
                nc.gpsimd.collective_compute(
                    "AllToAll", mybir.AluOpType.bypass,
                    replica_groups=[list(range(NCORES))],
                    ins=[a2a_in[h][b].opt()], outs=[a2a_out[h][b].opt()],
                )

            for qb in range(4):
                ph1_block(4 + qb)
                ph2_group(1, 0, qb)
            a2a(1, 0)
            for qb in range(4):
                ph2_group(1, 1, qb)
            a2a(1, 1)
            att_od = aap.tile([128, 8, 512], BF16, name="att_od")
            for j in range(NCORES):
                nc.gpsimd.dma_start(att_od[:, j, 0:256], a2a_out[1][0][j])
                nc.gpsimd.dma_start(att_od[:, j, 256:512], a2a_out[1][1][j])
            for qb in range(4):
                ph2_group(0, 0, qb)
            a2a(0, 0)
            for qb in range(4):
                ph2_group(0, 1, qb)
            a2a(0, 1)
            att_ev = aap.tile([128, 8, 512], BF16, name="att_ev")
            for j in range(NCORES):
                nc.sync.dma_start(att_ev[:, j, 0:256], a2a_out[0][0][j])
                nc.sync.dma_start(att_ev[:, j, 256:512], a2a_out[0][1][j])

            # ================= Phase 3: output projection =================
            # Pass 1 (odd heads, from the early A2A) runs while the final
            # A2A is in flight; partials spill to SBUF.  Pass 2 adds the
            # even heads and stores.
            def wo_load(n):
                n0 = n * 512
                tiles = []
                for wq4 in range(8):
                    wt_ = med.tile([128, 2, 512], BF16, tag="med",
                                   name=f"wo{wq4}")
                    nc.sync.dma_start(
                        wt_[:], woT_v[:, 2 * wq4:2 * wq4 + 2, n0:n0 + 512])
                    tiles.append(wt_)
                return tiles

            def half_mms(psO, att, kh0, wo_t, m):
                for j in range(8):
                    kh = 2 * j + kh0
                    nc.tensor.matmul(
                        psO[:],
                        att[:, j, m * 128:(m + 1) * 128],
                        wo_t[kh // 2][:, kh % 2, :],
                        start=(j == 0), stop=(j == 7),
                    )

            wo_ts = []
            for n in range(4):
                wo_t = wo_load(n)
                wo_ts.append(wo_t)
                for m in range(4):
                    pool, tag = ((psacc, "ps_a") if m % 2 == 0 else
                                 (psS, "ps_s"))
                    psO = pool.tile([128, 512], F32, tag=tag, name="ps_o")
                    half_mms(psO, att_od, 1, wo_t, m)
                    with nc.allow_low_precision(reason="bf16 partials"):
                        nc.scalar.copy(oev[:, 4 * n + m, :], psO[:])
            # Even-head pass split by batch-half: m=0,1 token columns
            # arrive with A2A(0,0) one collective earlier, so that half
            # releases mid-attention and only m=2,3 trails the final A2A.
            # (Viable now that w_o stays resident — the v11 attempt died
            # on reload-DMA pacing.)
            for half in range(2):
                for n in range(4):
                    wo_t = wo_ts[n]
                    oevfs = {}
                    for m in (2 * half, 2 * half + 1):
                        oevf = wkp.tile([128, 512], F32, tag="wk",
                                        name="oevf")
                        nc.scalar.copy(oevf[:], oev[:, 4 * n + m, :])
                        oevfs[m] = oevf
                    for m in (2 * half, 2 * half + 1):
                        pool, tag = ((psacc, "ps_a") if m % 2 == 0 else
                                     (psS, "ps_s"))
                        psO = pool.tile([128, 512], F32, tag=tag,
                                        name="ps_o")
                        half_mms(psO, att_ev, 0, wo_t, m)
                        oout = wkp.tile([128, 512], F32, tag="wk",
                                        name="oout")
                        nc.vector.tensor_add(oout[:], psO[:],
                                             oevfs[m][:])
                        nc.sync.dma_start(out[m * 128:(m + 1) * 128,
                                              n * 512:(n + 1) * 512],
                                          oout[:])

    nc.compile()
    return nc


def shard_inputs(cos, sin, hidden_states, w_qkv, w_o):
    """Host-side resharding into per-core input maps (pure data movement +
    layout transposes + dtype casts; no arithmetic on the model data)."""
    hs = np.ascontiguousarray(hidden_states.astype(np.float32))
    hT = np.ascontiguousarray(hs.reshape(TOK, HID).T).astype(NP_BF16)
    cosTt = np.ascontiguousarray(
        np.tile(cos.astype(np.float32).T, (1, B))).astype(NP_BF16)
    sT = sin.astype(np.float32).T                               # [128, 2048]
    sinST = np.concatenate([-sT[:64], sT[64:]], axis=0)
    sinST = np.ascontiguousarray(
        np.tile(sinST, (1, B))).astype(NP_BF16)                 # [128, 4096]
    woT = np.ascontiguousarray(w_o.astype(np.float32).T).astype(NP_BF16)
    ident = np.eye(128, dtype=np.float32).astype(NP_BF16)
    ones_col = np.ones((128, 1), np.float32).astype(NP_BF16)
    ones_row = np.ones((1, 128), np.float32).astype(NP_BF16)

    in_maps = []
    for c in range(NCORES):
        rows = [w_qkv[2 * c * D:(2 * c + 2) * D],
                w_qkv[(H + c) * D:(H + c + 1) * D],
                w_qkv[(H + HKV + c) * D:(H + HKV + c + 1) * D]]
        wq_c = np.concatenate(rows, axis=0).astype(np.float32)  # [512, 2048]
        wqT_c = np.ascontiguousarray(wq_c.T).astype(NP_BF16)    # [2048, 512]
        in_maps.append({
            "hT": hT, "wqT": wqT_c, "woT": woT,
            "cosT": cosTt, "sinST": sinST, "ident": ident,
            "ones_col": ones_col, "ones_row": ones_row,
        })
    return in_maps


_cached_nc = None


def kernel(cos, sin, hidden_states, w_qkv, w_o, _trace=False):
    global _cached_nc
    if _cached_nc is None:
        _cached_nc = build()
    nc = _cached_nc
    in_maps = shard_inputs(cos, sin, hidden_states, w_qkv, w_o)
    res = run_bass_kernel_spmd(nc, in_maps, core_ids=list(range(NCORES)),
                               trace=_trace)
    full = np.empty((B, S, HID), np.float32)
    for c in range(NCORES):
        p = res.results[c]["out"]
        full[0, 256 * c:256 * c + 256] = p[0:256]
        full[1, 256 * c:256 * c + 256] = p[256:512]
    out = np.ascontiguousarray(full)
    if _trace:
        return out, res
    return out
